# revision 74
# baseline (speedup 1.0000x reference)
"""CFConv (SchNet continuous-filter convolution) on 8 Trainium2 NeuronCores.

Reference computation (per atom i, neighbor slot k):
    W[i,k,:]  = ssp(dRexp[i,k,:] @ W1 + b1) @ W2 + b2       (filter network)
    C[i,k]    = (dR[i,k] <= 5.0)                            (hard cutoff)
    y         = x @ W_in2f                                  (atom embeddings)
    out[i,:]  = ssp( sum_k C*mask*W[i,k,:]*y[nbh[i,k],:] @ W_f2out + b_f2out )
    where ssp(v) = softplus(v) - log(2) = ln(0.5*e^v + 0.5)

Sharding: atoms split across 8 cores (1250 each).

Design decisions (vs. the original on-device-gather kernel):

1. HOST PRE-GATHER.  y = x @ W_in2f is one cheap host matmul; the per-edge
   expansion y[nbh[i,k],:] is a numpy take.  The device receives a dense
   pre-gathered feature-major fp32 tensor ygT[f, e] per core and performs no
   data-dependent access.  (The on-device dma_gather burned ~370us/core of
   serialized gpsimd descriptor generation plus 23.6MB of 512B random HBM
   packets -- the dominant cost of the old kernel.)

2. FP32-CLASS PRECISION EVERYWHERE.  The metric is
   max |err|/max(|expected|,1e-2); near-zero outputs need ~2e-4 ABSOLUTE
   accuracy.  Empirically every 16-bit (or tf32=float32r) stage alone
   measures 0.1-0.3 on this metric, so yg, h1s, products and sums stay fp32
   and mm2/f2out use plain fp32 matmuls.  mm1 keeps fp32-class precision at
   bf16 speed via a K-stacked hi/lo expansion: W1 and dRexp are split into
   bf16 hi+lo parts on the host and the four product terms are stacked along
   the contraction dim (K = 4*25 = 100 <= 128) of ONE bf16 matmul -- the
   systolic array's K-sum performs the exact hi/lo expansion, and matmul
   time depends only on columns, so the 4x K is free (1 cyc/col vs 4).

3. VALID-COUNT BUCKETING.  Atoms are host-sorted by their valid-neighbor
   count rounded up to a multiple of 2; each bucket kk processes only kk
   neighbor slots per atom instead of a uniform worst-case 36, cutting edge
   volume (and with it every engine's work + DMA bytes) by ~25%.  All 8
   cores run one SPMD program, so bucket capacities are the max over cores;
   pad chunks are all-zero and their outputs are ignored by the host unpack.

4. PIPELINED EMISSION.  Engines are in-order, so the instruction stream is
   software-pipelined: mm1(i+1) is emitted before mm2(i) (the PE never waits
   for the ACT exp/ln chain), f2out chunks are emitted a few pairs after
   their zT columns complete (never heading the PE queue before the k-sum
   lands), and the f2out PSUM is drained to SBUF by the DVE immediately so
   its bank never starves the main loop.

Device pipeline per bucket (feature-major: filters on partitions, edges on
the free dim, edge order e = atom-major (a, k), chunks padded to 512 columns
so each chunk fills exactly one fp32 PSUM bank):
  mm1:  h1T[f, e]  = W1^T @ dRexpT[g, e]       (K-stacked bf16 hi/lo, 1 MM)
  ACT:  u = exp(h1T + b1)                      (PSUM -> SBUF, per 1024 cols)
  ACT:  h1sT = ln(0.5*u + 0.5) = ssp(h1T)      (exact, shared exp/ln table)
  mm2:  WT[h, e]   = W2^T @ h1sT               (W2 stationary, fp32)
  DVE:  prodT[h,e] = WT_psum * ygT             (masked/padded edges have yg=0)
  DVE:  zT[h, a]   = k-slot sum (one strided tensor_reduce per slab)
  f2out: outT[o,:] = ssp(Wf^T @ zT + bf)       (matmul + exp + ln, inline)
Host transposes outT back to [atoms, features] and undoes the bucket sort.

Both Exp and Ln live in the "natural_log_exp_and_others" ACT table set; the
table chooser is patched so no per-instruction activation-table reloads
happen.
"""

import numpy as np
import ml_dtypes
from contextlib import ExitStack

BF16_NP = ml_dtypes.bfloat16

import concourse.bass as bass
import concourse.bacc as bacc
import concourse.mybir as mybir
import concourse.tile as tile

F32 = mybir.dt.float32
BF16 = mybir.dt.bfloat16
AOP = mybir.AluOpType
ACTF = mybir.ActivationFunctionType
AXIS = mybir.AxisListType

# ---- geometry (hardcoded for nn_CFConv_13245679141058) ----
N_ATOMS = 10000
K = 48                    # neighbors per atom
NIN = NF = NOUT = 128
NG = 25                   # gaussians
NCORES = 8
A_CORE = N_ATOMS // NCORES        # 1250 real atoms per core
CH = 512                          # columns per chunk (= one fp32 PSUM bank)
SLAB_CH = 4                       # chunks per DMA slab
R_CUTOFF = 5.0
LOG2 = float(np.log(2.0))


def build_nc(sig):
    """sig: tuple of (kk, nch) per bucket; nch even, chunk = 512 cols holding
    a_ch = 512//kk atoms of kk slots each (tail cols zero)."""
    ecols = sum(nch * CH for _, nch in sig)
    a_pad = sum(nch * (CH // kk) for kk, nch in sig)

    nc = bacc.Bacc()

    ygT_d = nc.declare_dram_parameter("ygT", [NF, ecols], F32, isOutput=False)
    # dRexp and W1 are shipped as bf16 hi/lo pairs.  mm1 = W1^T @ dRexp in
    # fp32 precision is computed as ONE bf16 matmul with the contraction dim
    # stacked 4x (K = 4*25 = 100 <= 128): rows [dh; dh; dl; dl] against
    # stationary rows [w1h; w1l; w1h; w1l].  The systolic array sums all K
    # rows, which IS the exact hi/lo product expansion -- full fp32-class
    # precision at bf16 streaming speed (1 cyc/col vs 4 for fp32), one
    # weight load, one instruction.  Matmul time depends only on columns,
    # so the 4x K is free.
    dre_d = nc.declare_dram_parameter("dreS", [3 * NG, ecols], BF16,
                                      isOutput=False)
    # packed constants: one bf16 tensor (stacked w1) and one fp32
    # (w2 | wf | b1 | bf) -> 2 DMAs at kernel start
    cb_d = nc.declare_dram_parameter("constb", [3 * NG, NF], BF16,
                                     isOutput=False)
    cf_d = nc.declare_dram_parameter("constf", [NF, 2 * NF + 2], F32,
                                     isOutput=False)
    outT_d = nc.declare_dram_parameter("outT", [NOUT, a_pad], F32, isOutput=True)

    with tile.TileContext(nc) as tc, ExitStack() as ctx:
        const = ctx.enter_context(tc.tile_pool(name="const", bufs=1))
        sb_yg = ctx.enter_context(tc.tile_pool(name="yg", bufs=5))
        sb_dre = ctx.enter_context(tc.tile_pool(name="dre", bufs=5))
        sb_u = ctx.enter_context(tc.tile_pool(name="u", bufs=3))
        sb_h = ctx.enter_context(tc.tile_pool(name="h1s", bufs=3))
        sb_p = ctx.enter_context(tc.tile_pool(name="prod", bufs=3))
        sb_p2 = ctx.enter_context(tc.tile_pool(name="pfold", bufs=3))
        sb_z = ctx.enter_context(tc.tile_pool(name="z", bufs=1))
        sb_o = ctx.enter_context(tc.tile_pool(name="f2o", bufs=2))
        psH = ctx.enter_context(tc.tile_pool(name="psH", bufs=2, space="PSUM"))
        psW = ctx.enter_context(tc.tile_pool(name="psW", bufs=2, space="PSUM"))

        # ---- constants (2 packed DMAs; the fp32 pack is deferred until
        # after slab 0's loads so the first mm1's inputs arrive first) ----
        w1s_sb = const.tile([3 * NG, NF], BF16)
        nc.sync.dma_start(w1s_sb[:], cb_d[:, :])
        cf_sb = const.tile([NF, 2 * NF + 2], F32)
        w2_sb = cf_sb[:, :NF]
        wf_sb = cf_sb[:, NF : 2 * NF]
        b1_sb = cf_sb[:, 2 * NF : 2 * NF + 1]
        bf_sb = cf_sb[:, 2 * NF + 1 :]
        half_sb = const.tile([128, 1], F32)
        nc.vector.memset(half_sb[:], 0.5)

        zT = sb_z.tile([NF, a_pad], F32)
        SC = SLAB_CH * CH

        # flatten the (bucket, slab, pair) structure so emission can be
        # software-pipelined across slab/bucket boundaries
        slabs = []
        ecoff = zoff = 0
        first_slab = True
        for kk, nch in sig:
            a_ch = CH // kk
            s0 = 0
            while s0 < nch:
                # the very first slab is kept small so the first mm1 starts
                # as soon as ~75KB (not ~300KB) has landed
                sc = min(1 if first_slab else SLAB_CH, nch - s0)
                first_slab = False
                slabs.append(
                    {"kk": kk, "a_ch": a_ch, "sc": sc,
                     "ecoff": ecoff, "zoff": zoff, "tiles": None}
                )
                ecoff += sc * CH
                zoff += sc * a_ch
                s0 += sc
        # chunk groups of width <= 2 (one psH tile / ACT instruction each)
        pairs = []
        for si, sl in enumerate(slabs):
            c0 = 0
            while c0 < sl["sc"]:
                w = min(2, sl["sc"] - c0)
                pairs.append((si, c0, w))
                c0 += w

        def tiles(si):
            sl = slabs[si]
            if sl["tiles"] is None:
                sc = sl["sc"]
                # dre first (mm1 needs it immediately); yg rides the scalar
                # engine's separate HWDGE ring and is only needed at product
                # time
                d_sl = sb_dre.tile([3 * NG, SC], BF16, tag="dre")
                nc.sync.dma_start(
                    d_sl[:, : sc * CH],
                    dre_d[:, sl["ecoff"] : sl["ecoff"] + sc * CH],
                )
                yg_sl = sb_yg.tile([NF, SC], F32, tag="yg")
                nc.scalar.dma_start(
                    yg_sl[:, : sc * CH],
                    ygT_d[:, sl["ecoff"] : sl["ecoff"] + sc * CH],
                )
                u_sl = sb_u.tile([NF, SC], F32, tag="u")
                h_sl = sb_h.tile([NF, SC], F32, tag="h1s")
                p_sl = sb_p.tile([NF, SC], F32, tag="prod")
                sl["tiles"] = (yg_sl, d_sl, u_sl, h_sl, p_sl)
            return sl["tiles"]

        def front(i):
            """mm1 group + exp + ln for pairs[i]."""
            si, c0, w = pairs[i]
            _, d_sl, u_sl, h_sl, _ = tiles(si)
            h1_ps = psH.tile([NF, 2 * CH], F32, tag="mm1")
            for j in range(w):
                c = c0 + j
                nc.tensor.matmul(
                    h1_ps[:, j * CH : (j + 1) * CH],
                    lhsT=w1s_sb[:],
                    rhs=d_sl[:, c * CH : (c + 1) * CH],
                    start=True,
                    stop=True,
                )
            # ssp(v) = ln(0.5*e^v + 0.5), shared exp/ln table set
            nc.scalar.activation(
                u_sl[:, c0 * CH : (c0 + w) * CH],
                h1_ps[:, : w * CH],
                ACTF.Exp,
                bias=b1_sb,
            )
            nc.scalar.activation(
                h_sl[:, c0 * CH : (c0 + w) * CH],
                u_sl[:, c0 * CH : (c0 + w) * CH],
                ACTF.Ln,
                bias=half_sb[:, :1],
                scale=0.5,
            )

        f2state = {"off": 0, "done": 0}

        def f2out_ready(final=False):
            """Emit f2out chunks whose zT columns completed a while ago (the
            lag keeps the in-order PE queue from stalling on the enabling
            k-sum still in flight on the DVE)."""
            while f2state["off"] < a_pad and (
                f2state["off"] + CH + 200 <= f2state["done"]
                or (final and f2state["done"] == a_pad)
            ):
                off = f2state["off"]
                n = min(CH, a_pad - off)
                o_ps = psW.tile([NOUT, 2 * CH], F32, tag="mm2",
                                name=f"f2_{off}")
                nc.tensor.matmul(
                    o_ps[:, :n], lhsT=wf_sb, rhs=zT[:, off : off + n],
                    start=True, stop=True,
                )
                # copy PSUM->SBUF on DVE right away so the psW buffer frees
                # quickly (the ACT queue may lag; holding the bank starves
                # the main loop's mm2)
                o32 = sb_o.tile([NOUT, CH], F32, tag="o32")
                nc.vector.tensor_copy(o32[:, :n], o_ps[:, :n])
                u2 = sb_o.tile([NOUT, CH], F32, tag="u2")
                nc.scalar.activation(u2[:, :n], o32[:, :n], ACTF.Exp,
                                     bias=bf_sb)
                o_sb = sb_o.tile([NOUT, CH], F32, tag="o")
                nc.scalar.activation(
                    o_sb[:, :n], u2[:, :n], ACTF.Ln,
                    bias=half_sb[:, :1], scale=0.5,
                )
                nc.sync.dma_start(outT_d[:, off : off + n], o_sb[:, :n])
                f2state["off"] = off + n

        def back(i):
            """mm2 group + product for pairs[i]; slab k-sum after its last
            group."""
            si, c0, w = pairs[i]
            sl = slabs[si]
            yg_sl, _, _, h_sl, p_sl = tiles(si)
            w_ps = psW.tile([NF, 2 * CH], F32, tag="mm2")
            for j in range(w):
                c = c0 + j
                nc.tensor.matmul(
                    w_ps[:, j * CH : (j + 1) * CH],
                    lhsT=w2_sb,
                    rhs=h_sl[:, c * CH : (c + 1) * CH],
                    start=True,
                    stop=True,
                )
            nc.vector.tensor_tensor(
                p_sl[:, c0 * CH : (c0 + w) * CH],
                w_ps[:, : w * CH],
                yg_sl[:, c0 * CH : (c0 + w) * CH],
                AOP.mult,
            )
            if c0 + w == sl["sc"]:
                sc, a_ch, kk = sl["sc"], sl["a_ch"], sl["kk"]
                # strided k-sum: p_sl as [f, chunk, atom, k]; pad tails
                # skipped by the AP.  (Offloading a pre-fold to gpsimd was
                # tried and measured ~15us SLOWER: Pool-engine elementwise
                # ops are well below DVE throughput and the fold sits in the
                # product->reduce critical chain.)
                nc.vector.tensor_reduce(
                    zT[:, sl["zoff"] : sl["zoff"] + sc * a_ch],
                    p_sl[:, : sc * CH]
                    .rearrange("f (c x) -> f c x", c=sc)[:, :, : a_ch * kk]
                    .rearrange("f c (a k) -> f c a k", k=kk),
                    axis=AXIS.X,
                    op=AOP.add,
                )
                f2state["done"] = sl["zoff"] + sc * a_ch

        # software-pipelined emission: the PE sees mm1(i+1) before mm2(i),
        # so it never idles while the ACT exp/ln chain for pair i finishes
        SKEW = 1
        tiles(0)                      # slab-0 loads queue before constf
        nc.sync.dma_start(cf_sb[:], cf_d[:, :])
        for si in range(1, min(3, len(slabs))):
            tiles(si)                 # prefetch the next slabs' streams
        for i in range(min(SKEW, len(pairs))):
            front(i)
        for i in range(len(pairs)):
            if i + SKEW < len(pairs):
                front(i + SKEW)
            back(i)
            # lag f2out ~3 pairs behind the watermark so its PE matmul never
            # heads the queue before its inputs are reduced
            if i % 3 == 0:
                f2out_ready()
        f2out_ready(final=True)
        assert f2state["off"] == a_pad    # f2out fully emitted

    # Pin Exp/Ln (and Copy/Identity) to the single shared table set so the
    # table chooser never inserts mid-kernel activation-table reloads.
    orig_tables = bacc.get_activation_tables

    def _one_set_tables(arch):
        t = orig_tables(arch)
        keep = "natural_log_exp_and_others"
        assert keep in t and ACTF.Exp in t[keep] and ACTF.Ln in t[keep]
        for name, funcs in t.items():
            if name != keep:
                for f in (ACTF.Exp, ACTF.Ln, ACTF.Copy, ACTF.Identity):
                    funcs.discard(f)
        return t

    bacc.get_activation_tables = _one_set_tables
    try:
        nc.compile()
    finally:
        bacc.get_activation_tables = orig_tables
    return nc


_NC_CACHE = {}


def _get_nc(sig):
    if sig not in _NC_CACHE:
        _NC_CACHE[sig] = build_nc(sig)
    return _NC_CACHE[sig]


def _make_sig(validF):
    """Shared SPMD bucket signature: (kk, nch) per bucket, kk ascending.
    nch = max chunk count over cores, rounded up to even.  Levels whose
    largest per-core population fills < 2 chunks are merged upward; a
    trailing sparse level bumps the last bucket's kk instead (so no atom
    ever lands in a bucket smaller than its valid count)."""
    v = validF.sum(1).astype(np.int64).reshape(NCORES, A_CORE)
    kk4 = np.clip(((v + 1) // 2) * 2, 2, K)
    levels = sorted(set(kk4.ravel().tolist()))
    counts = {kk: (kk4 == kk).sum(axis=1) for kk in levels}
    buckets = []                       # [kk, per-core n]
    carry = np.zeros(NCORES, np.int64)
    for i, kk in enumerate(levels):
        n = counts[kk] + carry
        if i + 1 < len(levels) and n.max() < 3 * (CH // kk):
            carry = n
            continue
        buckets.append([kk, n])
        carry = np.zeros(NCORES, np.int64)
    if carry.max() > 0:
        if buckets:
            buckets[-1][0] = levels[-1]
            buckets[-1][1] = buckets[-1][1] + carry
        else:
            buckets = [[levels[-1], carry]]
    sig = []
    for kk, n in buckets:
        a_ch = CH // kk
        sig.append((kk, int(-(-int(n.max()) // a_ch))))
    return tuple(sig), kk4


def make_in_maps(x, dR, dR_expanded, pairwise_mask, neighbors_idx,
                 W1, b1, W2, b2, W_in2f, W_f2out, b_f2out):
    x = np.asarray(x, np.float32)
    dR = np.asarray(dR, np.float32)
    dR_expanded = np.asarray(dR_expanded, np.float32)
    pairwise_mask = np.asarray(pairwise_mask, np.float32)
    neighbors_idx = np.asarray(neighbors_idx, np.int64)

    # atom embeddings + cutoff/mask folded into the host-side gather
    y = x @ np.asarray(W_in2f, np.float32)                  # [N, F]
    validF = (dR <= R_CUTOFF) & (pairwise_mask != 0.0)
    sig, kk4 = _make_sig(validF)
    sig_kks = np.array([kk for kk, _ in sig], np.int64)
    ecols = sum(nch * CH for _, nch in sig)
    a_pad = sum(nch * (CH // kk) for kk, nch in sig)

    w1f = np.asarray(W1, np.float32)
    w1h = w1f.astype(BF16_NP)
    w1l = (w1f - w1h.astype(np.float32)).astype(BF16_NP)
    # K-stacked hi/lo expansion: rows pair with dreS rows [dh; dh; dl; dl]
    constb = np.concatenate([w1h, w1l, w1h], axis=0)
    constf = np.concatenate(
        [
            np.asarray(W2, np.float32),
            np.asarray(W_f2out, np.float32),
            np.asarray(b1, np.float32).reshape(NF, 1),
            np.asarray(b_f2out, np.float32).reshape(NOUT, 1),
        ],
        axis=1,
    )
    common = {"constb": constb, "constf": constf}

    in_maps = []
    slots = []                       # per core: z-column slot of each atom
    for m in range(NCORES):
        sl = slice(m * A_CORE, (m + 1) * A_CORE)
        v = validF[sl]
        # each atom goes to the first bucket with kk >= its rounded count
        abkt = np.searchsorted(sig_kks, kk4[m])
        order = np.argsort(abkt, kind="stable")

        ygT = np.zeros((NF, ecols), np.float32)
        dreS = np.zeros((3 * NG, ecols), BF16_NP)
        slot = np.zeros(A_CORE, np.int64)

        ecoff = 0
        zoff = 0
        for bi, (kk, nch) in enumerate(sig):
            a_ch = CH // kk
            atoms = order[abkt[order] == bi]
            n = len(atoms)
            vb = v[atoms][:, :]                       # [n, K] valid masks
            perm = np.argsort(~vb, axis=1, kind="stable")[:, :kk]
            v_s = np.take_along_axis(vb, perm, 1)
            idx_s = np.take_along_axis(neighbors_idx[sl][atoms], perm, 1)
            dre_s = np.take_along_axis(
                dR_expanded[sl][atoms], perm[:, :, None], 1
            )
            n_pad = nch * a_ch
            yg = np.zeros((n_pad, kk, NF), np.float32)
            yg[:n] = np.where(v_s[..., None], y[idx_s], 0.0)
            dre = np.zeros((n_pad, kk, NG), np.float32)
            dre[:n] = dre_s
            # chunk layout: [nch, 512] cols; first a_ch*kk real, tail zero
            blk = ygT[:, ecoff : ecoff + nch * CH].reshape(NF, nch, CH)
            blk[:, :, : a_ch * kk] = (
                yg.reshape(nch, a_ch * kk, NF).transpose(2, 0, 1)
            )
            dreT = np.ascontiguousarray(
                dre.reshape(nch, a_ch * kk, NG).transpose(2, 0, 1)
            )
            dh = dreT.astype(BF16_NP)
            dl = (dreT - dh.astype(np.float32)).astype(BF16_NP)
            blk = dreS[:, ecoff : ecoff + nch * CH].reshape(3 * NG, nch, CH)
            blk[:NG, :, : a_ch * kk] = dh
            blk[NG : 2 * NG, :, : a_ch * kk] = dh
            blk[2 * NG :, :, : a_ch * kk] = dl
            slot[atoms] = zoff + np.arange(n)
            ecoff += nch * CH
            zoff += n_pad

        slots.append(slot)
        in_maps.append({**common, "ygT": ygT, "dreS": dreS})
    return in_maps, sig, slots


def kernel(**inputs) -> np.ndarray:
    from concourse.bass_utils import run_bass_kernel_spmd

    _check_b2(inputs["b2"])
    in_maps, sig, slots = make_in_maps(**inputs)
    nc = _get_nc(sig)
    res = run_bass_kernel_spmd(nc, in_maps, list(range(NCORES)))
    outs = []
    for m in range(NCORES):
        outT = np.asarray(res.results[m]["outT"])       # [NOUT, a_pad]
        outs.append(np.ascontiguousarray(outT.T[slots[m]]))
    return np.concatenate(outs, axis=0)


# b2 handling note: reference adds b2 after the second filter matmul.  In this
# problem b2 == 0; the general case would fold b2 into the product stage (the
# extra term is b2[h] * sum_k yg[i,k,h], computable host-side).  Assert so a
# non-zero b2 cannot silently give wrong results.
def _check_b2(b2):
    assert np.all(np.asarray(b2) == 0.0), "kernel assumes b2 == 0"


# revision 76
# speedup vs baseline: 1.0175x; 1.0175x over previous
"""CFConv (SchNet continuous-filter convolution) on 8 Trainium2 NeuronCores.

Reference computation (per atom i, neighbor slot k):
    W[i,k,:]  = ssp(dRexp[i,k,:] @ W1 + b1) @ W2 + b2       (filter network)
    C[i,k]    = (dR[i,k] <= 5.0)                            (hard cutoff)
    y         = x @ W_in2f                                  (atom embeddings)
    out[i,:]  = ssp( sum_k C*mask*W[i,k,:]*y[nbh[i,k],:] @ W_f2out + b_f2out )
    where ssp(v) = softplus(v) - log(2) = ln(0.5*e^v + 0.5)

Sharding: atoms split across 8 cores (1250 each).

Design decisions (vs. the original on-device-gather kernel):

1. HOST PRE-GATHER.  y = x @ W_in2f is one cheap host matmul; the per-edge
   expansion y[nbh[i,k],:] is a numpy take.  The device receives a dense
   pre-gathered feature-major fp32 tensor ygT[f, e] per core and performs no
   data-dependent access.  (The on-device dma_gather burned ~370us/core of
   serialized gpsimd descriptor generation plus 23.6MB of 512B random HBM
   packets -- the dominant cost of the old kernel.)

2. FP32-CLASS PRECISION EVERYWHERE.  The metric is
   max |err|/max(|expected|,1e-2); near-zero outputs need ~2e-4 ABSOLUTE
   accuracy.  Empirically every 16-bit (or tf32=float32r) stage alone
   measures 0.1-0.3 on this metric, so yg, h1s, products and sums stay fp32
   and mm2/f2out use plain fp32 matmuls.  mm1 keeps fp32-class precision at
   bf16 speed via a K-stacked hi/lo expansion: W1 and dRexp are split into
   bf16 hi+lo parts on the host and the four product terms are stacked along
   the contraction dim (K = 4*25 = 100 <= 128) of ONE bf16 matmul -- the
   systolic array's K-sum performs the exact hi/lo expansion, and matmul
   time depends only on columns, so the 4x K is free (1 cyc/col vs 4).

3. VALID-COUNT BUCKETING.  Atoms are host-sorted by their valid-neighbor
   count rounded up to a multiple of 2; each bucket kk processes only kk
   neighbor slots per atom instead of a uniform worst-case 36, cutting edge
   volume (and with it every engine's work + DMA bytes) by ~25%.  All 8
   cores run one SPMD program, so bucket capacities are the max over cores;
   pad chunks are all-zero and their outputs are ignored by the host unpack.

4. PIPELINED EMISSION.  Engines are in-order, so the instruction stream is
   software-pipelined: mm1(i+1) is emitted before mm2(i) (the PE never waits
   for the ACT exp/ln chain), f2out chunks are emitted a few pairs after
   their zT columns complete (never heading the PE queue before the k-sum
   lands), and the f2out PSUM is drained to SBUF by the DVE immediately so
   its bank never starves the main loop.

Device pipeline per bucket (feature-major: filters on partitions, edges on
the free dim, edge order e = atom-major (a, k), chunks padded to 512 columns
so each chunk fills exactly one fp32 PSUM bank):
  mm1:  h1T[f, e]  = W1^T @ dRexpT[g, e]       (K-stacked bf16 hi/lo, 1 MM)
  ACT:  u = exp(h1T + b1)                      (PSUM -> SBUF, per 1024 cols)
  ACT:  h1sT = ln(0.5*u + 0.5) = ssp(h1T)      (exact, shared exp/ln table)
  mm2:  WT[h, e]   = W2^T @ h1sT               (W2 stationary, fp32)
  DVE:  prodT[h,e] = WT_psum * ygT             (masked/padded edges have yg=0)
  DVE:  zT[h, a]   = k-slot sum (one strided tensor_reduce per slab)
  f2out: outT[o,:] = ssp(Wf^T @ zT + bf)       (matmul + exp + ln, inline)
Host transposes outT back to [atoms, features] and undoes the bucket sort.

Both Exp and Ln live in the "natural_log_exp_and_others" ACT table set; the
table chooser is patched so no per-instruction activation-table reloads
happen.
"""

import numpy as np
import ml_dtypes
from contextlib import ExitStack

BF16_NP = ml_dtypes.bfloat16

import concourse.bass as bass
import concourse.bacc as bacc
import concourse.mybir as mybir
import concourse.tile as tile

F32 = mybir.dt.float32
BF16 = mybir.dt.bfloat16
AOP = mybir.AluOpType
ACTF = mybir.ActivationFunctionType
AXIS = mybir.AxisListType

# ---- geometry (hardcoded for nn_CFConv_13245679141058) ----
N_ATOMS = 10000
K = 48                    # neighbors per atom
NIN = NF = NOUT = 128
NG = 25                   # gaussians
NCORES = 8
A_CORE = N_ATOMS // NCORES        # 1250 real atoms per core
CH = 512                          # columns per chunk (= one fp32 PSUM bank)
SLAB_CH = 4                       # chunks per DMA slab
R_CUTOFF = 5.0
LOG2 = float(np.log(2.0))


def build_nc(sig):
    """sig: tuple of (kk, nch) per bucket; nch even, chunk = 512 cols holding
    a_ch = 512//kk atoms of kk slots each (tail cols zero)."""
    ecols = sum(nch * CH for _, nch in sig)
    a_pad = sum(nch * (CH // kk) for kk, nch in sig)

    nc = bacc.Bacc()

    ygT_d = nc.declare_dram_parameter("ygT", [NF, ecols], F32, isOutput=False)
    # dRexp and W1 are shipped as bf16 hi/lo pairs.  mm1 = W1^T @ dRexp in
    # fp32 precision is computed as ONE bf16 matmul with the contraction dim
    # stacked 4x (K = 4*25 = 100 <= 128): rows [dh; dh; dl; dl] against
    # stationary rows [w1h; w1l; w1h; w1l].  The systolic array sums all K
    # rows, which IS the exact hi/lo product expansion -- full fp32-class
    # precision at bf16 streaming speed (1 cyc/col vs 4 for fp32), one
    # weight load, one instruction.  Matmul time depends only on columns,
    # so the 4x K is free.
    dre_d = nc.declare_dram_parameter("dreS", [3 * NG, ecols], BF16,
                                      isOutput=False)
    # packed constants: one bf16 tensor (stacked w1) and one fp32
    # (w2 | wf | b1 | bf) -> 2 DMAs at kernel start
    cb_d = nc.declare_dram_parameter("constb", [3 * NG, NF], BF16,
                                     isOutput=False)
    cf_d = nc.declare_dram_parameter("constf", [NF, 2 * NF + 2], F32,
                                     isOutput=False)
    outT_d = nc.declare_dram_parameter("outT", [NOUT, a_pad], F32, isOutput=True)

    with tile.TileContext(nc) as tc, ExitStack() as ctx:
        const = ctx.enter_context(tc.tile_pool(name="const", bufs=1))
        sb_yg = ctx.enter_context(tc.tile_pool(name="yg", bufs=5))
        sb_dre = ctx.enter_context(tc.tile_pool(name="dre", bufs=5))
        sb_u = ctx.enter_context(tc.tile_pool(name="u", bufs=3))
        sb_h = ctx.enter_context(tc.tile_pool(name="h1s", bufs=3))
        sb_p = ctx.enter_context(tc.tile_pool(name="prod", bufs=3))
        sb_p2 = ctx.enter_context(tc.tile_pool(name="pfold", bufs=3))
        sb_z = ctx.enter_context(tc.tile_pool(name="z", bufs=1))
        sb_o = ctx.enter_context(tc.tile_pool(name="f2o", bufs=2))
        psH = ctx.enter_context(tc.tile_pool(name="psH", bufs=2, space="PSUM"))
        psW = ctx.enter_context(tc.tile_pool(name="psW", bufs=2, space="PSUM"))

        # ---- constants (2 packed DMAs; the fp32 pack is deferred until
        # after slab 0's loads so the first mm1's inputs arrive first) ----
        w1s_sb = const.tile([3 * NG, NF], BF16)
        nc.sync.dma_start(w1s_sb[:], cb_d[:, :])
        cf_sb = const.tile([NF, 2 * NF + 2], F32)
        w2_sb = cf_sb[:, :NF]
        wf_sb = cf_sb[:, NF : 2 * NF]
        b1_sb = cf_sb[:, 2 * NF : 2 * NF + 1]
        bf_sb = cf_sb[:, 2 * NF + 1 :]
        half_sb = const.tile([128, 1], F32)
        nc.vector.memset(half_sb[:], 0.5)

        zT = sb_z.tile([NF, a_pad], F32)
        SC = SLAB_CH * CH

        # flatten the (bucket, slab, pair) structure so emission can be
        # software-pipelined across slab/bucket boundaries
        slabs = []
        ecoff = zoff = 0
        first_slab = True
        for kk, nch in sig:
            a_ch = CH // kk
            s0 = 0
            while s0 < nch:
                # the very first slab is kept small so the first mm1 starts
                # as soon as ~150KB (not ~300KB) has landed
                sc = min(2 if first_slab else SLAB_CH, nch - s0)
                first_slab = False
                slabs.append(
                    {"kk": kk, "a_ch": a_ch, "sc": sc,
                     "ecoff": ecoff, "zoff": zoff, "tiles": None}
                )
                ecoff += sc * CH
                zoff += sc * a_ch
                s0 += sc
        # chunk groups of width <= 2 (one psH tile / ACT instruction each)
        pairs = []
        for si, sl in enumerate(slabs):
            c0 = 0
            while c0 < sl["sc"]:
                w = min(2, sl["sc"] - c0)
                pairs.append((si, c0, w))
                c0 += w

        def tiles(si):
            sl = slabs[si]
            if sl["tiles"] is None:
                sc = sl["sc"]
                # dre first (mm1 needs it immediately); yg rides the scalar
                # engine's separate HWDGE ring and is only needed at product
                # time
                d_sl = sb_dre.tile([3 * NG, SC], BF16, tag="dre")
                nc.sync.dma_start(
                    d_sl[:, : sc * CH],
                    dre_d[:, sl["ecoff"] : sl["ecoff"] + sc * CH],
                )
                yg_sl = sb_yg.tile([NF, SC], F32, tag="yg")
                nc.scalar.dma_start(
                    yg_sl[:, : sc * CH],
                    ygT_d[:, sl["ecoff"] : sl["ecoff"] + sc * CH],
                )
                u_sl = sb_u.tile([NF, SC], F32, tag="u")
                h_sl = sb_h.tile([NF, SC], F32, tag="h1s")
                p_sl = sb_p.tile([NF, SC], F32, tag="prod")
                sl["tiles"] = (yg_sl, d_sl, u_sl, h_sl, p_sl)
            return sl["tiles"]

        def front(i):
            """mm1 group + exp + ln for pairs[i]."""
            si, c0, w = pairs[i]
            _, d_sl, u_sl, h_sl, _ = tiles(si)
            h1_ps = psH.tile([NF, 2 * CH], F32, tag="mm1")
            for j in range(w):
                c = c0 + j
                nc.tensor.matmul(
                    h1_ps[:, j * CH : (j + 1) * CH],
                    lhsT=w1s_sb[:],
                    rhs=d_sl[:, c * CH : (c + 1) * CH],
                    start=True,
                    stop=True,
                )
            # ssp(v) = ln(0.5*e^v + 0.5), shared exp/ln table set
            nc.scalar.activation(
                u_sl[:, c0 * CH : (c0 + w) * CH],
                h1_ps[:, : w * CH],
                ACTF.Exp,
                bias=b1_sb,
            )
            nc.scalar.activation(
                h_sl[:, c0 * CH : (c0 + w) * CH],
                u_sl[:, c0 * CH : (c0 + w) * CH],
                ACTF.Ln,
                bias=half_sb[:, :1],
                scale=0.5,
            )

        f2state = {"off": 0, "done": 0}

        def f2out_ready(final=False):
            """Emit f2out chunks whose zT columns completed a while ago (the
            lag keeps the in-order PE queue from stalling on the enabling
            k-sum still in flight on the DVE)."""
            while f2state["off"] < a_pad and (
                f2state["off"] + CH <= f2state["done"]
                or (final and f2state["done"] == a_pad)
            ):
                off = f2state["off"]
                n = min(CH, a_pad - off)
                o_ps = psW.tile([NOUT, 2 * CH], F32, tag="mm2",
                                name=f"f2_{off}")
                nc.tensor.matmul(
                    o_ps[:, :n], lhsT=wf_sb, rhs=zT[:, off : off + n],
                    start=True, stop=True,
                )
                # copy PSUM->SBUF on DVE right away so the psW buffer frees
                # quickly (the ACT queue may lag; holding the bank starves
                # the main loop's mm2)
                o32 = sb_o.tile([NOUT, CH], F32, tag="o32")
                nc.vector.tensor_copy(o32[:, :n], o_ps[:, :n])
                u2 = sb_o.tile([NOUT, CH], F32, tag="u2")
                nc.scalar.activation(u2[:, :n], o32[:, :n], ACTF.Exp,
                                     bias=bf_sb)
                o_sb = sb_o.tile([NOUT, CH], F32, tag="o")
                nc.scalar.activation(
                    o_sb[:, :n], u2[:, :n], ACTF.Ln,
                    bias=half_sb[:, :1], scale=0.5,
                )
                nc.sync.dma_start(outT_d[:, off : off + n], o_sb[:, :n])
                f2state["off"] = off + n

        def back(i):
            """mm2 group + product for pairs[i]; slab k-sum after its last
            group."""
            si, c0, w = pairs[i]
            sl = slabs[si]
            yg_sl, _, _, h_sl, p_sl = tiles(si)
            w_ps = psW.tile([NF, 2 * CH], F32, tag="mm2")
            for j in range(w):
                c = c0 + j
                nc.tensor.matmul(
                    w_ps[:, j * CH : (j + 1) * CH],
                    lhsT=w2_sb,
                    rhs=h_sl[:, c * CH : (c + 1) * CH],
                    start=True,
                    stop=True,
                )
            nc.vector.tensor_tensor(
                p_sl[:, c0 * CH : (c0 + w) * CH],
                w_ps[:, : w * CH],
                yg_sl[:, c0 * CH : (c0 + w) * CH],
                AOP.mult,
            )
            if c0 + w == sl["sc"]:
                sc, a_ch, kk = sl["sc"], sl["a_ch"], sl["kk"]
                # strided k-sum: p_sl as [f, chunk, atom, k]; pad tails
                # skipped by the AP.  (Offloading a pre-fold to gpsimd was
                # tried and measured ~15us SLOWER: Pool-engine elementwise
                # ops are well below DVE throughput and the fold sits in the
                # product->reduce critical chain.)
                nc.vector.tensor_reduce(
                    zT[:, sl["zoff"] : sl["zoff"] + sc * a_ch],
                    p_sl[:, : sc * CH]
                    .rearrange("f (c x) -> f c x", c=sc)[:, :, : a_ch * kk]
                    .rearrange("f c (a k) -> f c a k", k=kk),
                    axis=AXIS.X,
                    op=AOP.add,
                )
                f2state["done"] = sl["zoff"] + sc * a_ch

        # software-pipelined emission: the PE sees mm1(i+1) before mm2(i),
        # so it never idles while the ACT exp/ln chain for pair i finishes
        SKEW = 1
        tiles(0)                      # slab-0 loads queue before constf
        nc.sync.dma_start(cf_sb[:], cf_d[:, :])
        for si in range(1, min(3, len(slabs))):
            tiles(si)                 # prefetch the next slabs' streams
        for i in range(min(SKEW, len(pairs))):
            front(i)
        for i in range(len(pairs)):
            if i + SKEW < len(pairs):
                front(i + SKEW)
            back(i)
            # lag f2out ~3 pairs behind the watermark so its PE matmul never
            # heads the queue before its inputs are reduced
            if i % 3 == 0:
                f2out_ready()
        f2out_ready(final=True)
        assert f2state["off"] == a_pad    # f2out fully emitted

    # Pin Exp/Ln (and Copy/Identity) to the single shared table set so the
    # table chooser never inserts mid-kernel activation-table reloads.
    orig_tables = bacc.get_activation_tables

    def _one_set_tables(arch):
        t = orig_tables(arch)
        keep = "natural_log_exp_and_others"
        assert keep in t and ACTF.Exp in t[keep] and ACTF.Ln in t[keep]
        for name, funcs in t.items():
            if name != keep:
                for f in (ACTF.Exp, ACTF.Ln, ACTF.Copy, ACTF.Identity):
                    funcs.discard(f)
        return t

    bacc.get_activation_tables = _one_set_tables
    try:
        nc.compile()
    finally:
        bacc.get_activation_tables = orig_tables
    return nc


_NC_CACHE = {}


def _get_nc(sig):
    if sig not in _NC_CACHE:
        _NC_CACHE[sig] = build_nc(sig)
    return _NC_CACHE[sig]


def _make_sig(validF):
    """Shared SPMD bucket signature: (kk, nch) per bucket, kk ascending.
    nch = max chunk count over cores, rounded up to even.  Levels whose
    largest per-core population fills < 2 chunks are merged upward; a
    trailing sparse level bumps the last bucket's kk instead (so no atom
    ever lands in a bucket smaller than its valid count)."""
    v = validF.sum(1).astype(np.int64).reshape(NCORES, A_CORE)
    kk4 = np.clip(((v + 1) // 2) * 2, 2, K)
    levels = sorted(set(kk4.ravel().tolist()))
    counts = {kk: (kk4 == kk).sum(axis=1) for kk in levels}
    buckets = []                       # [kk, per-core n]
    carry = np.zeros(NCORES, np.int64)
    for i, kk in enumerate(levels):
        n = counts[kk] + carry
        if i + 1 < len(levels) and n.max() < 3 * (CH // kk):
            carry = n
            continue
        buckets.append([kk, n])
        carry = np.zeros(NCORES, np.int64)
    if carry.max() > 0:
        if buckets:
            buckets[-1][0] = levels[-1]
            buckets[-1][1] = buckets[-1][1] + carry
        else:
            buckets = [[levels[-1], carry]]
    sig = []
    for kk, n in buckets:
        a_ch = CH // kk
        sig.append((kk, int(-(-int(n.max()) // a_ch))))
    return tuple(sig), kk4


def make_in_maps(x, dR, dR_expanded, pairwise_mask, neighbors_idx,
                 W1, b1, W2, b2, W_in2f, W_f2out, b_f2out):
    x = np.asarray(x, np.float32)
    dR = np.asarray(dR, np.float32)
    dR_expanded = np.asarray(dR_expanded, np.float32)
    pairwise_mask = np.asarray(pairwise_mask, np.float32)
    neighbors_idx = np.asarray(neighbors_idx, np.int64)

    # atom embeddings + cutoff/mask folded into the host-side gather
    y = x @ np.asarray(W_in2f, np.float32)                  # [N, F]
    validF = (dR <= R_CUTOFF) & (pairwise_mask != 0.0)
    sig, kk4 = _make_sig(validF)
    sig_kks = np.array([kk for kk, _ in sig], np.int64)
    ecols = sum(nch * CH for _, nch in sig)
    a_pad = sum(nch * (CH // kk) for kk, nch in sig)

    w1f = np.asarray(W1, np.float32)
    w1h = w1f.astype(BF16_NP)
    w1l = (w1f - w1h.astype(np.float32)).astype(BF16_NP)
    # K-stacked hi/lo expansion: rows pair with dreS rows [dh; dh; dl; dl]
    constb = np.concatenate([w1h, w1l, w1h], axis=0)
    constf = np.concatenate(
        [
            np.asarray(W2, np.float32),
            np.asarray(W_f2out, np.float32),
            np.asarray(b1, np.float32).reshape(NF, 1),
            np.asarray(b_f2out, np.float32).reshape(NOUT, 1),
        ],
        axis=1,
    )
    common = {"constb": constb, "constf": constf}

    in_maps = []
    slots = []                       # per core: z-column slot of each atom
    for m in range(NCORES):
        sl = slice(m * A_CORE, (m + 1) * A_CORE)
        v = validF[sl]
        # each atom goes to the first bucket with kk >= its rounded count
        abkt = np.searchsorted(sig_kks, kk4[m])
        order = np.argsort(abkt, kind="stable")

        ygT = np.zeros((NF, ecols), np.float32)
        dreS = np.zeros((3 * NG, ecols), BF16_NP)
        slot = np.zeros(A_CORE, np.int64)

        ecoff = 0
        zoff = 0
        for bi, (kk, nch) in enumerate(sig):
            a_ch = CH // kk
            atoms = order[abkt[order] == bi]
            n = len(atoms)
            vb = v[atoms][:, :]                       # [n, K] valid masks
            perm = np.argsort(~vb, axis=1, kind="stable")[:, :kk]
            v_s = np.take_along_axis(vb, perm, 1)
            idx_s = np.take_along_axis(neighbors_idx[sl][atoms], perm, 1)
            dre_s = np.take_along_axis(
                dR_expanded[sl][atoms], perm[:, :, None], 1
            )
            n_pad = nch * a_ch
            yg = np.zeros((n_pad, kk, NF), np.float32)
            yg[:n] = np.where(v_s[..., None], y[idx_s], 0.0)
            dre = np.zeros((n_pad, kk, NG), np.float32)
            dre[:n] = dre_s
            # chunk layout: [nch, 512] cols; first a_ch*kk real, tail zero
            blk = ygT[:, ecoff : ecoff + nch * CH].reshape(NF, nch, CH)
            blk[:, :, : a_ch * kk] = (
                yg.reshape(nch, a_ch * kk, NF).transpose(2, 0, 1)
            )
            dreT = np.ascontiguousarray(
                dre.reshape(nch, a_ch * kk, NG).transpose(2, 0, 1)
            )
            dh = dreT.astype(BF16_NP)
            dl = (dreT - dh.astype(np.float32)).astype(BF16_NP)
            blk = dreS[:, ecoff : ecoff + nch * CH].reshape(3 * NG, nch, CH)
            blk[:NG, :, : a_ch * kk] = dh
            blk[NG : 2 * NG, :, : a_ch * kk] = dh
            blk[2 * NG :, :, : a_ch * kk] = dl
            slot[atoms] = zoff + np.arange(n)
            ecoff += nch * CH
            zoff += n_pad

        slots.append(slot)
        in_maps.append({**common, "ygT": ygT, "dreS": dreS})
    return in_maps, sig, slots


def kernel(**inputs) -> np.ndarray:
    from concourse.bass_utils import run_bass_kernel_spmd

    _check_b2(inputs["b2"])
    in_maps, sig, slots = make_in_maps(**inputs)
    nc = _get_nc(sig)
    res = run_bass_kernel_spmd(nc, in_maps, list(range(NCORES)))
    outs = []
    for m in range(NCORES):
        outT = np.asarray(res.results[m]["outT"])       # [NOUT, a_pad]
        outs.append(np.ascontiguousarray(outT.T[slots[m]]))
    return np.concatenate(outs, axis=0)


# b2 handling note: reference adds b2 after the second filter matmul.  In this
# problem b2 == 0; the general case would fold b2 into the product stage (the
# extra term is b2[h] * sum_k yg[i,k,h], computable host-side).  Assert so a
# non-zero b2 cannot silently give wrong results.
def _check_b2(b2):
    assert np.all(np.asarray(b2) == 0.0), "kernel assumes b2 == 0"


# revision 78
# speedup vs baseline: 1.0342x; 1.0164x over previous
"""CFConv (SchNet continuous-filter convolution) on 8 Trainium2 NeuronCores.

Reference computation (per atom i, neighbor slot k):
    W[i,k,:]  = ssp(dRexp[i,k,:] @ W1 + b1) @ W2 + b2       (filter network)
    C[i,k]    = (dR[i,k] <= 5.0)                            (hard cutoff)
    y         = x @ W_in2f                                  (atom embeddings)
    out[i,:]  = ssp( sum_k C*mask*W[i,k,:]*y[nbh[i,k],:] @ W_f2out + b_f2out )
    where ssp(v) = softplus(v) - log(2) = ln(0.5*e^v + 0.5)

Sharding: atoms split across 8 cores (1250 each).

Design decisions (vs. the original on-device-gather kernel):

1. HOST PRE-GATHER.  y = x @ W_in2f is one cheap host matmul; the per-edge
   expansion y[nbh[i,k],:] is a numpy take.  The device receives a dense
   pre-gathered feature-major fp32 tensor ygT[f, e] per core and performs no
   data-dependent access.  (The on-device dma_gather burned ~370us/core of
   serialized gpsimd descriptor generation plus 23.6MB of 512B random HBM
   packets -- the dominant cost of the old kernel.)

2. FP32-CLASS PRECISION EVERYWHERE.  The metric is
   max |err|/max(|expected|,1e-2); near-zero outputs need ~2e-4 ABSOLUTE
   accuracy.  Empirically every 16-bit (or tf32=float32r) stage alone
   measures 0.1-0.3 on this metric, so yg, h1s, products and sums stay fp32
   and mm2/f2out use plain fp32 matmuls.  mm1 keeps fp32-class precision at
   bf16 speed via a K-stacked hi/lo expansion: W1 and dRexp are split into
   bf16 hi+lo parts on the host and the four product terms are stacked along
   the contraction dim (K = 4*25 = 100 <= 128) of ONE bf16 matmul -- the
   systolic array's K-sum performs the exact hi/lo expansion, and matmul
   time depends only on columns, so the 4x K is free (1 cyc/col vs 4).

3. VALID-COUNT BUCKETING.  Atoms are host-sorted by their valid-neighbor
   count rounded up to a multiple of 2; each bucket kk processes only kk
   neighbor slots per atom instead of a uniform worst-case 36, cutting edge
   volume (and with it every engine's work + DMA bytes) by ~25%.  All 8
   cores run one SPMD program, so bucket capacities are the max over cores;
   pad chunks are all-zero and their outputs are ignored by the host unpack.

4. PIPELINED EMISSION.  Engines are in-order, so the instruction stream is
   software-pipelined: mm1(i+1) is emitted before mm2(i) (the PE never waits
   for the ACT exp/ln chain), f2out chunks are emitted a few pairs after
   their zT columns complete (never heading the PE queue before the k-sum
   lands), and the f2out PSUM is drained to SBUF by the DVE immediately so
   its bank never starves the main loop.

Device pipeline per bucket (feature-major: filters on partitions, edges on
the free dim, edge order e = atom-major (a, k), chunks padded to 512 columns
so each chunk fills exactly one fp32 PSUM bank):
  mm1:  h1T[f, e]  = W1^T @ dRexpT[g, e]       (K-stacked bf16 hi/lo, 1 MM)
  ACT:  u = exp(h1T + b1)                      (PSUM -> SBUF, per 1024 cols)
  ACT:  h1sT = ln(0.5*u + 0.5) = ssp(h1T)      (exact, shared exp/ln table)
  mm2:  WT[h, e]   = W2^T @ h1sT               (W2 stationary, fp32)
  DVE:  prodT[h,e] = WT_psum * ygT             (masked/padded edges have yg=0)
  DVE:  zT[h, a]   = k-slot sum (one strided tensor_reduce per slab)
  f2out: outT[o,:] = ssp(Wf^T @ zT + bf)       (matmul + exp + ln, inline)
Host transposes outT back to [atoms, features] and undoes the bucket sort.

Both Exp and Ln live in the "natural_log_exp_and_others" ACT table set; the
table chooser is patched so no per-instruction activation-table reloads
happen.
"""

import numpy as np
import ml_dtypes
from contextlib import ExitStack

BF16_NP = ml_dtypes.bfloat16

import concourse.bass as bass
import concourse.bacc as bacc
import concourse.mybir as mybir
import concourse.tile as tile

F32 = mybir.dt.float32
BF16 = mybir.dt.bfloat16
AOP = mybir.AluOpType
ACTF = mybir.ActivationFunctionType
AXIS = mybir.AxisListType

# ---- geometry (hardcoded for nn_CFConv_13245679141058) ----
N_ATOMS = 10000
K = 48                    # neighbors per atom
NIN = NF = NOUT = 128
NG = 25                   # gaussians
NCORES = 8
A_CORE = N_ATOMS // NCORES        # 1250 real atoms per core
CH = 512                          # columns per chunk (= one fp32 PSUM bank)
SLAB_CH = 4                       # chunks per DMA slab
R_CUTOFF = 5.0
LOG2 = float(np.log(2.0))


def build_nc(sig):
    """sig: tuple of (kk, nch) per bucket; nch even, chunk = 512 cols holding
    a_ch = 512//kk atoms of kk slots each (tail cols zero)."""
    ecols = sum(nch * CH for _, nch in sig)
    a_pad = sum(nch * (CH // kk) for kk, nch in sig)

    nc = bacc.Bacc()

    ygT_d = nc.declare_dram_parameter("ygT", [NF, ecols], F32, isOutput=False)
    # dRexp and W1 are shipped as bf16 hi/lo pairs.  mm1 = W1^T @ dRexp in
    # fp32 precision is computed as ONE bf16 matmul with the contraction dim
    # stacked 4x (K = 4*25 = 100 <= 128): rows [dh; dh; dl; dl] against
    # stationary rows [w1h; w1l; w1h; w1l].  The systolic array sums all K
    # rows, which IS the exact hi/lo product expansion -- full fp32-class
    # precision at bf16 streaming speed (1 cyc/col vs 4 for fp32), one
    # weight load, one instruction.  Matmul time depends only on columns,
    # so the 4x K is free.
    dre_d = nc.declare_dram_parameter("dreS", [3 * NG, ecols], BF16,
                                      isOutput=False)
    # packed constants: one bf16 tensor (stacked w1) and one fp32
    # (w2 | wf | b1 | bf) -> 2 DMAs at kernel start
    cb_d = nc.declare_dram_parameter("constb", [3 * NG, NF], BF16,
                                     isOutput=False)
    cf_d = nc.declare_dram_parameter("constf", [NF, 2 * NF + 2], F32,
                                     isOutput=False)
    outT_d = nc.declare_dram_parameter("outT", [NOUT, a_pad], F32, isOutput=True)

    with tile.TileContext(nc) as tc, ExitStack() as ctx:
        const = ctx.enter_context(tc.tile_pool(name="const", bufs=1))
        sb_yg = ctx.enter_context(tc.tile_pool(name="yg", bufs=5))
        sb_dre = ctx.enter_context(tc.tile_pool(name="dre", bufs=5))
        sb_u = ctx.enter_context(tc.tile_pool(name="u", bufs=3))
        sb_h = ctx.enter_context(tc.tile_pool(name="h1s", bufs=3))
        sb_p = ctx.enter_context(tc.tile_pool(name="prod", bufs=3))
        sb_p2 = ctx.enter_context(tc.tile_pool(name="pfold", bufs=3))
        sb_z = ctx.enter_context(tc.tile_pool(name="z", bufs=1))
        sb_o = ctx.enter_context(tc.tile_pool(name="f2o", bufs=2))
        psH = ctx.enter_context(tc.tile_pool(name="psH", bufs=2, space="PSUM"))
        psW = ctx.enter_context(tc.tile_pool(name="psW", bufs=2, space="PSUM"))

        # ---- constants (2 packed DMAs; the fp32 pack is deferred until
        # after slab 0's loads so the first mm1's inputs arrive first) ----
        w1s_sb = const.tile([3 * NG, NF], BF16)
        nc.sync.dma_start(w1s_sb[:], cb_d[:, :])
        cf_sb = const.tile([NF, 2 * NF + 2], F32)
        w2_sb = cf_sb[:, :NF]
        wf_sb = cf_sb[:, NF : 2 * NF]
        b1_sb = cf_sb[:, 2 * NF : 2 * NF + 1]
        bf_sb = cf_sb[:, 2 * NF + 1 :]
        half_sb = const.tile([128, 1], F32)
        nc.vector.memset(half_sb[:], 0.5)

        zT = sb_z.tile([NF, a_pad], F32)
        SC = SLAB_CH * CH

        # flatten the (bucket, slab, pair) structure so emission can be
        # software-pipelined across slab/bucket boundaries
        slabs = []
        ecoff = zoff = 0
        first_slab = True
        for kk, nch in sig:
            a_ch = CH // kk
            s0 = 0
            while s0 < nch:
                # the very first slab is kept small so the first mm1 starts
                # as soon as ~150KB (not ~300KB) has landed
                sc = min(2 if first_slab else SLAB_CH, nch - s0)
                first_slab = False
                slabs.append(
                    {"kk": kk, "a_ch": a_ch, "sc": sc,
                     "ecoff": ecoff, "zoff": zoff, "tiles": None}
                )
                ecoff += sc * CH
                zoff += sc * a_ch
                s0 += sc
        # chunk groups of width <= 2 (one psH tile / ACT instruction each)
        pairs = []
        for si, sl in enumerate(slabs):
            c0 = 0
            while c0 < sl["sc"]:
                w = min(2, sl["sc"] - c0)
                pairs.append((si, c0, w))
                c0 += w

        def tiles(si):
            sl = slabs[si]
            if sl["tiles"] is None:
                sc = sl["sc"]
                # dre first (mm1 needs it immediately); yg rides the scalar
                # engine's separate HWDGE ring and is only needed at product
                # time
                d_sl = sb_dre.tile([3 * NG, SC], BF16, tag="dre")
                nc.sync.dma_start(
                    d_sl[:, : sc * CH],
                    dre_d[:, sl["ecoff"] : sl["ecoff"] + sc * CH],
                )
                yg_sl = sb_yg.tile([NF, SC], F32, tag="yg")
                nc.scalar.dma_start(
                    yg_sl[:, : sc * CH],
                    ygT_d[:, sl["ecoff"] : sl["ecoff"] + sc * CH],
                )
                u_sl = sb_u.tile([NF, SC], F32, tag="u")
                h_sl = sb_h.tile([NF, SC], F32, tag="h1s")
                p_sl = sb_p.tile([NF, SC], F32, tag="prod")
                sl["tiles"] = (yg_sl, d_sl, u_sl, h_sl, p_sl)
            return sl["tiles"]

        def front(i):
            """mm1 group + exp + ln for pairs[i].  All ops are narrowed to
            the chunk's real column count rk = a_ch*kk (the 512-col tail pad
            is never computed; downstream APs skip it)."""
            si, c0, w = pairs[i]
            sl = slabs[si]
            rk = sl["a_ch"] * sl["kk"]
            _, d_sl, u_sl, h_sl, _ = tiles(si)
            h1_ps = psH.tile([NF, 2 * CH], F32, tag="mm1")
            for j in range(w):
                c = c0 + j
                nc.tensor.matmul(
                    h1_ps[:, j * CH : j * CH + rk],
                    lhsT=w1s_sb[:],
                    rhs=d_sl[:, c * CH : c * CH + rk],
                    start=True,
                    stop=True,
                )

            def cview(t, off):
                return t[:, off * CH : (off + w) * CH].rearrange(
                    "f (c x) -> f c x", c=w
                )[:, :, :rk]

            # ssp(v) = ln(0.5*e^v + 0.5), shared exp/ln table set
            nc.scalar.activation(
                cview(u_sl, c0), cview(h1_ps, 0), ACTF.Exp, bias=b1_sb,
            )
            nc.scalar.activation(
                cview(h_sl, c0), cview(u_sl, c0), ACTF.Ln,
                bias=half_sb[:, :1], scale=0.5,
            )

        f2state = {"off": 0, "done": 0}

        def f2out_ready(final=False):
            """Emit f2out chunks whose zT columns completed a while ago (the
            lag keeps the in-order PE queue from stalling on the enabling
            k-sum still in flight on the DVE)."""
            while f2state["off"] < a_pad and (
                f2state["off"] + CH <= f2state["done"]
                or (final and f2state["done"] == a_pad)
            ):
                off = f2state["off"]
                n = min(CH, a_pad - off)
                o_ps = psW.tile([NOUT, 2 * CH], F32, tag="mm2",
                                name=f"f2_{off}")
                nc.tensor.matmul(
                    o_ps[:, :n], lhsT=wf_sb, rhs=zT[:, off : off + n],
                    start=True, stop=True,
                )
                # copy PSUM->SBUF on DVE right away so the psW buffer frees
                # quickly (the ACT queue may lag; holding the bank starves
                # the main loop's mm2)
                o32 = sb_o.tile([NOUT, CH], F32, tag="o32")
                nc.vector.tensor_copy(o32[:, :n], o_ps[:, :n])
                u2 = sb_o.tile([NOUT, CH], F32, tag="u2")
                nc.scalar.activation(u2[:, :n], o32[:, :n], ACTF.Exp,
                                     bias=bf_sb)
                o_sb = sb_o.tile([NOUT, CH], F32, tag="o")
                nc.scalar.activation(
                    o_sb[:, :n], u2[:, :n], ACTF.Ln,
                    bias=half_sb[:, :1], scale=0.5,
                )
                nc.sync.dma_start(outT_d[:, off : off + n], o_sb[:, :n])
                f2state["off"] = off + n

        def back(i):
            """mm2 group + product for pairs[i]; slab k-sum after its last
            group."""
            si, c0, w = pairs[i]
            sl = slabs[si]
            rk = sl["a_ch"] * sl["kk"]
            yg_sl, _, _, h_sl, p_sl = tiles(si)
            w_ps = psW.tile([NF, 2 * CH], F32, tag="mm2")
            for j in range(w):
                c = c0 + j
                nc.tensor.matmul(
                    w_ps[:, j * CH : j * CH + rk],
                    lhsT=w2_sb,
                    rhs=h_sl[:, c * CH : c * CH + rk],
                    start=True,
                    stop=True,
                )

            def cview(t, off):
                return t[:, off * CH : (off + w) * CH].rearrange(
                    "f (c x) -> f c x", c=w
                )[:, :, :rk]

            nc.vector.tensor_tensor(
                cview(p_sl, c0), cview(w_ps, 0), cview(yg_sl, c0), AOP.mult,
            )
            if c0 + w == sl["sc"]:
                sc, a_ch, kk = sl["sc"], sl["a_ch"], sl["kk"]
                # strided k-sum: p_sl as [f, chunk, atom, k]; pad tails
                # skipped by the AP.  (Offloading a pre-fold to gpsimd was
                # tried and measured ~15us SLOWER: Pool-engine elementwise
                # ops are well below DVE throughput and the fold sits in the
                # product->reduce critical chain.)
                nc.vector.tensor_reduce(
                    zT[:, sl["zoff"] : sl["zoff"] + sc * a_ch],
                    p_sl[:, : sc * CH]
                    .rearrange("f (c x) -> f c x", c=sc)[:, :, : a_ch * kk]
                    .rearrange("f c (a k) -> f c a k", k=kk),
                    axis=AXIS.X,
                    op=AOP.add,
                )
                f2state["done"] = sl["zoff"] + sc * a_ch

        # software-pipelined emission: the PE sees mm1(i+1) before mm2(i),
        # so it never idles while the ACT exp/ln chain for pair i finishes
        SKEW = 1
        tiles(0)                      # slab-0 loads queue before constf
        nc.sync.dma_start(cf_sb[:], cf_d[:, :])
        for si in range(1, min(3, len(slabs))):
            tiles(si)                 # prefetch the next slabs' streams
        for i in range(min(SKEW, len(pairs))):
            front(i)
        for i in range(len(pairs)):
            if i + SKEW < len(pairs):
                front(i + SKEW)
            back(i)
            # lag f2out ~3 pairs behind the watermark so its PE matmul never
            # heads the queue before its inputs are reduced
            if i % 3 == 0:
                f2out_ready()
        f2out_ready(final=True)
        assert f2state["off"] == a_pad    # f2out fully emitted

    # Pin Exp/Ln (and Copy/Identity) to the single shared table set so the
    # table chooser never inserts mid-kernel activation-table reloads.
    orig_tables = bacc.get_activation_tables

    def _one_set_tables(arch):
        t = orig_tables(arch)
        keep = "natural_log_exp_and_others"
        assert keep in t and ACTF.Exp in t[keep] and ACTF.Ln in t[keep]
        for name, funcs in t.items():
            if name != keep:
                for f in (ACTF.Exp, ACTF.Ln, ACTF.Copy, ACTF.Identity):
                    funcs.discard(f)
        return t

    bacc.get_activation_tables = _one_set_tables
    try:
        nc.compile()
    finally:
        bacc.get_activation_tables = orig_tables
    return nc


_NC_CACHE = {}


def _get_nc(sig):
    if sig not in _NC_CACHE:
        _NC_CACHE[sig] = build_nc(sig)
    return _NC_CACHE[sig]


def _make_sig(validF):
    """Shared SPMD bucket signature: (kk, nch) per bucket, kk ascending.
    nch = max chunk count over cores, rounded up to even.  Levels whose
    largest per-core population fills < 2 chunks are merged upward; a
    trailing sparse level bumps the last bucket's kk instead (so no atom
    ever lands in a bucket smaller than its valid count)."""
    v = validF.sum(1).astype(np.int64).reshape(NCORES, A_CORE)
    kk4 = np.clip(((v + 1) // 2) * 2, 2, K)
    levels = sorted(set(kk4.ravel().tolist()))
    counts = {kk: (kk4 == kk).sum(axis=1) for kk in levels}
    buckets = []                       # [kk, per-core n]
    carry = np.zeros(NCORES, np.int64)
    for i, kk in enumerate(levels):
        n = counts[kk] + carry
        if i + 1 < len(levels) and n.max() < 3 * (CH // kk):
            carry = n
            continue
        buckets.append([kk, n])
        carry = np.zeros(NCORES, np.int64)
    if carry.max() > 0:
        if buckets:
            buckets[-1][0] = levels[-1]
            buckets[-1][1] = buckets[-1][1] + carry
        else:
            buckets = [[levels[-1], carry]]
    sig = []
    for kk, n in buckets:
        a_ch = CH // kk
        sig.append((kk, int(-(-int(n.max()) // a_ch))))
    return tuple(sig), kk4


def make_in_maps(x, dR, dR_expanded, pairwise_mask, neighbors_idx,
                 W1, b1, W2, b2, W_in2f, W_f2out, b_f2out):
    x = np.asarray(x, np.float32)
    dR = np.asarray(dR, np.float32)
    dR_expanded = np.asarray(dR_expanded, np.float32)
    pairwise_mask = np.asarray(pairwise_mask, np.float32)
    neighbors_idx = np.asarray(neighbors_idx, np.int64)

    # atom embeddings + cutoff/mask folded into the host-side gather
    y = x @ np.asarray(W_in2f, np.float32)                  # [N, F]
    validF = (dR <= R_CUTOFF) & (pairwise_mask != 0.0)
    sig, kk4 = _make_sig(validF)
    sig_kks = np.array([kk for kk, _ in sig], np.int64)
    ecols = sum(nch * CH for _, nch in sig)
    a_pad = sum(nch * (CH // kk) for kk, nch in sig)

    w1f = np.asarray(W1, np.float32)
    w1h = w1f.astype(BF16_NP)
    w1l = (w1f - w1h.astype(np.float32)).astype(BF16_NP)
    # K-stacked hi/lo expansion: rows pair with dreS rows [dh; dh; dl; dl]
    constb = np.concatenate([w1h, w1l, w1h], axis=0)
    constf = np.concatenate(
        [
            np.asarray(W2, np.float32),
            np.asarray(W_f2out, np.float32),
            np.asarray(b1, np.float32).reshape(NF, 1),
            np.asarray(b_f2out, np.float32).reshape(NOUT, 1),
        ],
        axis=1,
    )
    common = {"constb": constb, "constf": constf}

    in_maps = []
    slots = []                       # per core: z-column slot of each atom
    for m in range(NCORES):
        sl = slice(m * A_CORE, (m + 1) * A_CORE)
        v = validF[sl]
        # each atom goes to the first bucket with kk >= its rounded count
        abkt = np.searchsorted(sig_kks, kk4[m])
        order = np.argsort(abkt, kind="stable")

        ygT = np.zeros((NF, ecols), np.float32)
        dreS = np.zeros((3 * NG, ecols), BF16_NP)
        slot = np.zeros(A_CORE, np.int64)

        ecoff = 0
        zoff = 0
        for bi, (kk, nch) in enumerate(sig):
            a_ch = CH // kk
            atoms = order[abkt[order] == bi]
            n = len(atoms)
            vb = v[atoms][:, :]                       # [n, K] valid masks
            perm = np.argsort(~vb, axis=1, kind="stable")[:, :kk]
            v_s = np.take_along_axis(vb, perm, 1)
            idx_s = np.take_along_axis(neighbors_idx[sl][atoms], perm, 1)
            dre_s = np.take_along_axis(
                dR_expanded[sl][atoms], perm[:, :, None], 1
            )
            n_pad = nch * a_ch
            yg = np.zeros((n_pad, kk, NF), np.float32)
            yg[:n] = np.where(v_s[..., None], y[idx_s], 0.0)
            dre = np.zeros((n_pad, kk, NG), np.float32)
            dre[:n] = dre_s
            # chunk layout: [nch, 512] cols; first a_ch*kk real, tail zero
            blk = ygT[:, ecoff : ecoff + nch * CH].reshape(NF, nch, CH)
            blk[:, :, : a_ch * kk] = (
                yg.reshape(nch, a_ch * kk, NF).transpose(2, 0, 1)
            )
            dreT = np.ascontiguousarray(
                dre.reshape(nch, a_ch * kk, NG).transpose(2, 0, 1)
            )
            dh = dreT.astype(BF16_NP)
            dl = (dreT - dh.astype(np.float32)).astype(BF16_NP)
            blk = dreS[:, ecoff : ecoff + nch * CH].reshape(3 * NG, nch, CH)
            blk[:NG, :, : a_ch * kk] = dh
            blk[NG : 2 * NG, :, : a_ch * kk] = dh
            blk[2 * NG :, :, : a_ch * kk] = dl
            slot[atoms] = zoff + np.arange(n)
            ecoff += nch * CH
            zoff += n_pad

        slots.append(slot)
        in_maps.append({**common, "ygT": ygT, "dreS": dreS})
    return in_maps, sig, slots


def kernel(**inputs) -> np.ndarray:
    from concourse.bass_utils import run_bass_kernel_spmd

    _check_b2(inputs["b2"])
    in_maps, sig, slots = make_in_maps(**inputs)
    nc = _get_nc(sig)
    res = run_bass_kernel_spmd(nc, in_maps, list(range(NCORES)))
    outs = []
    for m in range(NCORES):
        outT = np.asarray(res.results[m]["outT"])       # [NOUT, a_pad]
        outs.append(np.ascontiguousarray(outT.T[slots[m]]))
    return np.concatenate(outs, axis=0)


# b2 handling note: reference adds b2 after the second filter matmul.  In this
# problem b2 == 0; the general case would fold b2 into the product stage (the
# extra term is b2[h] * sum_k yg[i,k,h], computable host-side).  Assert so a
# non-zero b2 cannot silently give wrong results.
def _check_b2(b2):
    assert np.all(np.asarray(b2) == 0.0), "kernel assumes b2 == 0"
